# revision 1
# baseline (speedup 1.0000x reference)
"""GAT layer (gnn_message_passing) on 8 Trainium2 NeuronCores.

Strategy (dst-partitioned, replicated projection table):
  * Nodes padded to NPAD=50176; core p owns dst nodes [p*6272, (p+1)*6272)
    = 49 blocks of 128.
  * Every core computes the full projected table xp = x @ W.T into its DRAM
    as bf16, feature-permuted head-last (j = c*4+h) and pre-scaled by
    att_src (both folded into the projection weights).
  * Edges (+self loops) are bucketed per (core, dst-block) and split into two
    classes by src parity; gather index = src//2 (superrow of 1024B = 2 rows)
    so indices fit int16. Each (block, class) cell is padded to SUBT*128.
  * Per cell: dma_gather the 512B rows, one-hot(dst-slot), per-edge
    w = exp(leaky_relu(a_src + a_dst)); aggregate sum_e w*xp[src] and the
    denominator with one-hot matmuls into a per-block PSUM accumulator.
    a_src = row-sum per head of the pre-scaled gathered row; a_dst comes from
    a tiny per-block gather of the block's own rows (per-core indices) and a
    (att_dst/att_src)-weighted row-sum.
  * Within a block, dst slot r maps to node offset q: r = q//2 + 64*(q%2)
    (even nodes in slots 0..63, odd in 64..127); host unpermutes at assembly.
  * Finalize per block: normalize, undo att_src pre-scale, transpose, fused
    BN+bias affine + ReLU, final linear -> [6272, 64] per core; host
    reassembles [50000, 64] float32.
"""

import numpy as np
import ml_dtypes

BF16 = ml_dtypes.bfloat16

# ---- problem constants ----
N, E, F, H, C = 50000, 800000, 256, 4, 64
NEG_SLOPE = 0.2
BN_EPS = 1e-5
NCORES = 8
BLK = 128
NB = 49                 # dst blocks per core
OWN = NB * BLK          # 6272 dsts per core
NPAD = NCORES * OWN     # 50176
NT = NPAD // 128        # 392 projection tiles
NSUP = NPAD // 2        # 25088 superrows (int16-addressable)

# feature permutation: new index j = c*4 + h  <->  old index h*64 + c
_OLD_OF_NEW = (np.arange(F) % H) * C + (np.arange(F) // H)

LAST_EXEC_NS = None
LAST_RESULTS = None


def _prep_edges(edge_index):
    src = np.asarray(edge_index[0], dtype=np.int64)
    dst = np.asarray(edge_index[1], dtype=np.int64)
    src = np.concatenate([src, np.arange(N, dtype=np.int64)])
    dst = np.concatenate([dst, np.arange(N, dtype=np.int64)])

    core = dst // OWN
    dst_local = dst - core * OWN
    block = dst_local // BLK
    q = dst_local % BLK
    dst_slot = (q // 2 + 64 * (q % 2)).astype(np.float32)  # parity-permuted
    cls = (src % 2).astype(np.int64)
    gidx = (src // 2).astype(np.int64)                     # superrow index

    ncell_per_core = NB * 2
    cell = core * ncell_per_core + block * 2 + cls
    ncells = NCORES * ncell_per_core
    counts = np.bincount(cell, minlength=ncells).reshape(NCORES, ncell_per_core)
    # exact per-(block,cls) gather count (max over cores), 16-aligned;
    # compute shapes use the 128-aligned subtile count
    nie_list = [int(np.ceil(counts[:, ci].max() / 16)) * 16
                for ci in range(ncell_per_core)]
    subt_list = [(n + 127) // 128 for n in nie_list]
    ni_list = [s * 128 for s in subt_list]
    offs = np.zeros(ncell_per_core + 1, dtype=np.int64)
    np.cumsum(ni_list, out=offs[1:])
    TOT = int(offs[-1])

    order = np.argsort(cell, kind="stable")
    sorted_cell = cell[order]
    cell_starts = np.zeros(ncells + 1, dtype=np.int64)
    np.cumsum(counts.reshape(-1), out=cell_starts[1:])
    rank = np.arange(len(order)) - cell_starts[sorted_cell]
    ci_of = sorted_cell % ncell_per_core
    core_of = sorted_cell // ncell_per_core
    flat_pos = core_of * TOT + offs[ci_of] + rank

    gidx_pad = np.zeros(NCORES * TOT, dtype=np.int64)
    gidx_pad[flat_pos] = gidx[order]
    dstm_pad = np.full(NCORES * TOT, 200.0, dtype=np.float32)
    dstm_pad[flat_pos] = dst_slot[order]
    g3 = gidx_pad.reshape(NCORES, TOT)
    d3 = dstm_pad.reshape(NCORES, TOT)

    # wrapped gather indices [16, TOT//16] -> replicated x8 across partitions
    g = g3.astype(np.int16).reshape(NCORES, TOT // 16, 16)
    g = np.ascontiguousarray(g.transpose(0, 2, 1))
    idx_all = np.tile(g, (1, 8, 1))                    # [8, 128, TOT//16]

    # dst-slot stream in gather layout (edge i at [i%128, i//128]); ragged
    # per-chunk [S_ci * 128] -> [128, S_ci] slices concatenated along free
    dst_w = np.empty((NCORES, 128, TOT // 128), dtype=BF16)
    for ci in range(ncell_per_core):
        seg = d3[:, offs[ci]:offs[ci + 1]].reshape(NCORES, subt_list[ci], 128)
        dst_w[:, :, offs[ci] // 128:offs[ci + 1] // 128] = \
            seg.transpose(0, 2, 1).astype(BF16)
    dT_all = d3.astype(BF16)                           # [8, TOT] free-stream

    # upfront own-superrow gather indices: 3136 consecutive, padded to 3200
    NOWN = OWN // 2
    NOWNP = ((NOWN + 127) // 128) * 128
    own = np.full((NCORES, NOWNP), -1, dtype=np.int16)
    for p in range(NCORES):
        own[p, :NOWN] = np.arange(p * NOWN, (p + 1) * NOWN, dtype=np.int64)
    ow = own.reshape(NCORES, NOWNP // 16, 16)
    ow = np.ascontiguousarray(ow.transpose(0, 2, 1))
    own_all = np.tile(ow, (1, 8, 1))                   # [8, 128, NOWNP//16]

    return idx_all, dst_w, dT_all, own_all, (subt_list, nie_list)


def _prep_params(x, W, att_src, att_dst, gat_bias, bn_gamma, bn_beta,
                 bn_mean, bn_var, lin_W, lin_b):
    f32 = np.float32
    W = np.asarray(W, f32)
    att_src_f = np.asarray(att_src, f32).reshape(H * C)      # index h*64+c
    att_dst_f = np.asarray(att_dst, f32).reshape(H * C)

    wt = W.T                                                 # [in, out]
    wt_perm = wt[:, _OLD_OF_NEW] * att_src_f[_OLD_OF_NEW][None, :]
    # matmul rhs chunk k = wt_perm[k*128:(k+1)*128, :] (K rows = input feats)
    wt_ext = np.ascontiguousarray(wt_perm.reshape(2, 128, F)).astype(BF16)

    xT = np.zeros((F, NPAD), dtype=f32)
    xT[:, :N] = np.asarray(x, f32).T
    # [NT, 128 partitions, 2 k-chunks, 128 nodes]: partition-major so each
    # tile loads as one contiguous 64KB DMA with 512B per partition
    xT_t = np.ascontiguousarray(
        xT.reshape(2, 128, NT, 128).transpose(2, 1, 0, 3)).astype(BF16)

    att_inv = (1.0 / att_src_f[_OLD_OF_NEW]).astype(f32)
    att_inv_rep = np.tile(att_inv[None, :], (128, 1))
    ratio = (att_dst_f[_OLD_OF_NEW] / att_src_f[_OLD_OF_NEW]).astype(f32)
    ratio_rep = np.tile(ratio[None, :], (128, 1))

    bnscale = np.asarray(bn_gamma, f32) / np.sqrt(np.asarray(bn_var, f32) + BN_EPS)
    bnshift = ((np.asarray(gat_bias, f32) - np.asarray(bn_mean, f32)) * bnscale
               + np.asarray(bn_beta, f32))
    bnsc = np.ascontiguousarray(bnscale[_OLD_OF_NEW].reshape(2, 128).T)
    bnsh = np.ascontiguousarray(bnshift[_OLD_OF_NEW].reshape(2, 128).T)

    linw = np.asarray(lin_W, f32).T[_OLD_OF_NEW, :]
    linw_t = np.ascontiguousarray(linw.reshape(2, 128, 64)).astype(BF16)
    linb_rep = np.tile(np.asarray(lin_b, f32)[None, :], (128, 1))

    iota_row = np.tile(np.arange(128, dtype=np.float32)[None, :],
                       (128, 1)).astype(BF16)
    iota_p = np.arange(128, dtype=np.float32).reshape(128, 1).astype(BF16)
    ident_f32 = np.eye(128, dtype=np.float32)

    return dict(xT_t=xT_t, wt_ext=wt_ext, att_inv=att_inv_rep.astype(f32),
                ratio=ratio_rep.astype(f32), bnsc=bnsc.astype(f32),
                bnsh=bnsh.astype(f32), linw=linw_t, linb=linb_rep.astype(f32),
                iota=iota_row, iotap=iota_p, ident_f32=ident_f32)


def _build(subt_cfg):
    import concourse.bacc as bacc
    import concourse.mybir as mybir
    import concourse.tile as tile

    dt = mybir.dt
    subt_list, nie_list = subt_cfg
    NCH = NB * 2
    ni_list = [s * 128 for s in subt_list]
    offs = [0]
    for n in ni_list:
        offs.append(offs[-1] + n)
    TOT = offs[-1]
    SMAX = max(subt_list)
    NOWN = OWN // 2
    NOWNP = ((NOWN + 127) // 128) * 128

    nc = bacc.Bacc("TRN2", target_bir_lowering=False, debug=False,
                   enable_asserts=False, num_devices=NCORES)

    xT_in = nc.dram_tensor("xT_t", [NT, 128, 2, 128], dt.bfloat16, kind="ExternalInput")
    wt_in = nc.dram_tensor("wt_ext", [2, 128, 256], dt.bfloat16, kind="ExternalInput")
    attinv_in = nc.dram_tensor("att_inv", [128, 256], dt.float32, kind="ExternalInput")
    ratio_in = nc.dram_tensor("ratio", [128, 256], dt.float32, kind="ExternalInput")
    bnsc_in = nc.dram_tensor("bnsc", [128, 2], dt.float32, kind="ExternalInput")
    bnsh_in = nc.dram_tensor("bnsh", [128, 2], dt.float32, kind="ExternalInput")
    linw_in = nc.dram_tensor("linw", [2, 128, 64], dt.bfloat16, kind="ExternalInput")
    linb_in = nc.dram_tensor("linb", [128, 64], dt.float32, kind="ExternalInput")
    iota_in = nc.dram_tensor("iota", [128, 128], dt.bfloat16, kind="ExternalInput")
    iotap_in = nc.dram_tensor("iotap", [128, 1], dt.bfloat16, kind="ExternalInput")
    identf_in = nc.dram_tensor("ident_f32", [128, 128], dt.float32, kind="ExternalInput")
    idx_in = nc.dram_tensor("idx", [128, TOT // 16], dt.int16, kind="ExternalInput")
    dstm_in = nc.dram_tensor("dstm", [128, TOT // 128], dt.bfloat16, kind="ExternalInput")
    dstmT_in = nc.dram_tensor("dstmT", [TOT], dt.bfloat16, kind="ExternalInput")
    own_in = nc.dram_tensor("ownidx", [128, NOWNP // 16], dt.int16, kind="ExternalInput")
    out_dram = nc.dram_tensor("out", [OWN, 64], dt.float32, kind="ExternalOutput")

    with tile.TileContext(nc) as tc:
        with (
            tc.tile_pool(name="dram", bufs=1, space="DRAM") as dramp,
            tc.tile_pool(name="const", bufs=1) as constp,
        ):
            xp_table = dramp.tile([NPAD, 256], dt.bfloat16)
            # superrow views: [25088, 512] -> even/odd 256-col halves
            sup = xp_table[:].rearrange("(s two) f -> s (two f)", two=2)

            wt_sb = constp.tile([128, 2, 256], dt.bfloat16)
            for k in range(2):
                nc.sync.dma_start(out=wt_sb[:, k, :], in_=wt_in[k])

            # ---- phase A: projection ----
            with (
                tc.tile_pool(name="proj_sb", bufs=3) as psb,
                tc.tile_pool(name="proj_ps", bufs=3, space="PSUM") as pps,
            ):
                for ntile in range(NT):
                    xt = psb.tile([128, 2, 128], dt.bfloat16)
                    nc.sync.dma_start(out=xt[:], in_=xT_in[ntile])
                    ps = pps.tile([128, 256], dt.float32, space="PSUM")
                    nc.tensor.matmul(out=ps[:], lhsT=xt[:, 0, :],
                                     rhs=wt_sb[:, 0, :], start=True, stop=False)
                    nc.tensor.matmul(out=ps[:], lhsT=xt[:, 1, :],
                                     rhs=wt_sb[:, 1, :], start=False, stop=True)
                    xp_sb = psb.tile([128, 256], dt.bfloat16)
                    nc.vector.tensor_copy(out=xp_sb[:], in_=ps[:])
                    nc.scalar.dma_start(
                        out=xp_table[ntile * 128:(ntile + 1) * 128, :],
                        in_=xp_sb[:])

            # ---- phase B consts ----
            att_inv_sb = constp.tile([128, 256], dt.float32)
            nc.sync.dma_start(out=att_inv_sb[:], in_=attinv_in[:])
            ratio_sb = constp.tile([128, 256], dt.float32)
            nc.sync.dma_start(out=ratio_sb[:], in_=ratio_in[:])
            bnsc_sb = constp.tile([128, 2], dt.float32)
            nc.sync.dma_start(out=bnsc_sb[:], in_=bnsc_in[:])
            bnsh_sb = constp.tile([128, 2], dt.float32)
            nc.sync.dma_start(out=bnsh_sb[:], in_=bnsh_in[:])
            linw_sb = constp.tile([128, 2, 64], dt.bfloat16)
            for k in range(2):
                nc.sync.dma_start(out=linw_sb[:, k, :], in_=linw_in[k])
            linb_sb = constp.tile([128, 64], dt.float32)
            nc.sync.dma_start(out=linb_sb[:], in_=linb_in[:])
            iota_sb = constp.tile([128, 128], dt.bfloat16)
            nc.sync.dma_start(out=iota_sb[:], in_=iota_in[:])
            iotap_sb = constp.tile([128, 1], dt.bfloat16)
            nc.sync.dma_start(out=iotap_sb[:], in_=iotap_in[:])
            identf_sb = constp.tile([128, 128], dt.float32)
            nc.sync.dma_start(out=identf_sb[:], in_=identf_in[:])
            idx_sb = constp.tile([128, TOT // 16], dt.int16)
            nc.sync.dma_start(out=idx_sb[:], in_=idx_in[:])
            dstm_sb = constp.tile([128, TOT // 128], dt.bfloat16)
            nc.sync.dma_start(out=dstm_sb[:], in_=dstm_in[:])
            own_sb = constp.tile([128, NOWNP // 16], dt.int16)
            nc.sync.dma_start(out=own_sb[:], in_=own_in[:])


            # ---- phase B: per-block pipeline ----
            with (
                tc.tile_pool(name="gsb", bufs=5) as gsb,
                tc.tile_pool(name="osb", bufs=1) as osb,
                tc.tile_pool(name="msb", bufs=3) as msb,
                tc.tile_pool(name="ohsb", bufs=4) as ohsb,
                tc.tile_pool(name="fsb", bufs=2) as fsb,
                tc.tile_pool(name="aggps", bufs=3, space="PSUM") as aggps,
                tc.tile_pool(name="tps", bufs=2, space="PSUM") as tps,
                tc.tile_pool(name="adstps", bufs=2, space="PSUM") as adstps,
                tc.tile_pool(name="finps", bufs=1, space="PSUM") as finps,
            ):
                # ---- upfront: all own superrows -> a_dst table ----
                xo = osb.tile([128, NOWNP // 128, 512], dt.bfloat16, tag="xo")
                if NOWN < NOWNP:
                    nc.vector.memset(xo[:, NOWNP // 128 - 1, :], 0.0)
                for g0 in range(0, NOWNP // 128, 8):
                    nrem = min(NOWN - g0 * 128, 1024)
                    gs = (nrem + 127) // 128
                    nc.gpsimd.dma_gather(
                        out_ap=xo[:, g0:g0 + gs, :], in_ap=sup[:, 0:512],
                        idxs_ap=own_sb[:, g0 * 8:g0 * 8 + (nrem + 15) // 16],
                        num_idxs=nrem, num_idxs_reg=nrem,
                        elem_size=512, elem_step=512)
                # a_dst[p, par, slot, h] via ratio-weighted head sums
                NSLOT = NOWNP // 128
                adst_all = osb.tile([128, 2, NSLOT, 4], dt.float32, tag="adall")
                for s0 in range(0, NSLOT, 5):
                    sl = min(5, NSLOT - s0)
                    adt = osb.tile([128, 5, 2, 256], dt.bfloat16, tag="adt")
                    nc.vector.tensor_tensor(
                        out=adt[:, 0:sl, :, :],
                        in0=xo[:, s0:s0 + sl, :].rearrange(
                            "p s (two f) -> p s two f", two=2),
                        in1=ratio_sb[:, None, None, :].to_broadcast(
                            [128, sl, 2, 256]),
                        op=mybir.AluOpType.mult)
                    for par in range(2):
                        nc.vector.reduce_sum(
                            out=adst_all[:, par, s0:s0 + sl, :],
                            in_=adt[:, 0:sl, par, :].rearrange(
                                "p s (c h) -> p s h c", h=H),
                            axis=mybir.AxisListType.X)
                # shuffle to block-aligned layout (even blocks first)
                adst_ord = osb.tile([128, NB, 4], dt.float32, tag="adord")
                NEB = (NB + 1) // 2   # even blocks 0,2,..,48 -> 25
                NOB = NB // 2         # odd blocks 1,3,..,47 -> 24
                nc.sync.dma_start(out=adst_ord[0:64, 0:NEB, :],
                                  in_=adst_all[0:64, 0, 0:NEB, :])
                nc.sync.dma_start(out=adst_ord[64:128, 0:NEB, :],
                                  in_=adst_all[0:64, 1, 0:NEB, :])
                nc.sync.dma_start(out=adst_ord[0:64, NEB:NB, :],
                                  in_=adst_all[64:128, 0, 0:NOB, :])
                nc.sync.dma_start(out=adst_ord[64:128, NEB:NB, :],
                                  in_=adst_all[64:128, 1, 0:NOB, :])
                adst_bf = osb.tile([128, NB, 4], dt.bfloat16, tag="adbf")
                nc.vector.tensor_copy(out=adst_bf[:], in_=adst_ord[:])

                def bord(b):
                    return b // 2 if b % 2 == 0 else NEB + b // 2

                for b in range(NB):
                    agg = aggps.tile([128, 260], dt.float32, space="PSUM")
                    for cls in range(2):
                        ci = b * 2 + cls
                        S = subt_list[ci]
                        NI = ni_list[ci]
                        oE = offs[ci]           # edge offset
                        oW = oE // 16           # wrapped-idx col offset
                        oS = oE // 128          # dstm col offset
                        xg = gsb.tile([128, SMAX, 256], dt.bfloat16,
                                      tag=f"xg{cls}")
                        src_ap = sup[:, 0:256] if cls == 0 else sup[:, 256:512]
                        nie = nie_list[ci]
                        if nie < S * 128:
                            nc.vector.memset(xg[:, S - 1, :], 0.0)
                        for g0 in range(0, S, 8):
                            nrem = min(nie - g0 * 128, 1024)
                            if nrem <= 0:
                                break
                            gs = (nrem + 127) // 128
                            nc.gpsimd.dma_gather(
                                out_ap=xg[:, g0:g0 + gs, :], in_ap=src_ap,
                                idxs_ap=idx_sb[:, oW + g0 * 8:
                                               oW + g0 * 8 + (nrem + 15) // 16],
                                num_idxs=nrem, num_idxs_reg=nrem,
                                elem_size=256, elem_step=512)
                        # one-hot [edge_p, dst] and transposed [dst_p, edge]
                        oh = ohsb.tile([128, SMAX, 128], dt.bfloat16, tag="oh")
                        nc.vector.tensor_tensor(
                            out=oh[:, 0:S, :],
                            in0=dstm_sb[:, oS:oS + S, None].to_broadcast(
                                [128, S, 128]),
                            in1=iota_sb[:, None, :].to_broadcast(
                                [128, S, 128]),
                            op=mybir.AluOpType.is_equal)
                        dT = ohsb.tile([128, SMAX, 128], dt.bfloat16, tag="dT")
                        nc.scalar.dma_start(
                            out=dT[:, 0:S, :],
                            in_=dstmT_in[None, oE:oE + NI].to_broadcast(
                                [128, NI]))
                        ohT = ohsb.tile([128, SMAX, 128], dt.bfloat16, tag="ohT")
                        nc.vector.tensor_tensor(
                            out=ohT[:, 0:S, :], in0=dT[:, 0:S, :],
                            in1=iotap_sb[:, :, None].to_broadcast(
                                [128, S, 128]),
                            op=mybir.AluOpType.is_equal)
                        # a_src: head-wise row sums via 2x-mode tree adds
                        xg4 = xg[:, 0:S, :].rearrange(
                            "p t (c h) -> p t c h", h=H)
                        tr1 = msb.tile([128, SMAX, 32, 4], dt.bfloat16,
                                       tag="tr1")
                        nc.vector.tensor_tensor(
                            out=tr1[:, 0:S, :, :], in0=xg4[:, :, 0:32, :],
                            in1=xg4[:, :, 32:64, :], op=mybir.AluOpType.add)
                        tr2 = msb.tile([128, SMAX, 16, 4], dt.bfloat16,
                                       tag="tr2")
                        nc.vector.tensor_tensor(
                            out=tr2[:, 0:S, :, :], in0=tr1[:, 0:S, 0:16, :],
                            in1=tr1[:, 0:S, 16:32, :], op=mybir.AluOpType.add)
                        tr3 = msb.tile([128, SMAX, 8, 4], dt.bfloat16,
                                       tag="tr3")
                        nc.vector.tensor_tensor(
                            out=tr3[:, 0:S, :, :], in0=tr2[:, 0:S, 0:8, :],
                            in1=tr2[:, 0:S, 8:16, :], op=mybir.AluOpType.add)
                        asrc = msb.tile([128, SMAX, 4], dt.float32, tag="asrc")
                        nc.vector.reduce_sum(
                            out=asrc[:, 0:S, :],
                            in_=tr3[:, 0:S, :, :].rearrange(
                                "p t c h -> p t h c"),
                            axis=mybir.AxisListType.X)
                        # a_dst per edge via one-hot-transpose matmuls
                        adps = adstps.tile([128, SMAX, 4], dt.float32,
                                           space="PSUM")
                        for t in range(S):
                            nc.tensor.matmul(out=adps[:, t, :],
                                             lhsT=ohT[:, t, :],
                                             rhs=adst_bf[:, bord(b), :],
                                             start=True, stop=True)
                        # w = exp(leaky_relu(a_src + a_dst)) -> msg cols 256:260
                        ev = msb.tile([128, SMAX, 4], dt.float32, tag="ev")
                        nc.vector.tensor_tensor(out=ev[:, 0:S, :],
                                                in0=asrc[:, 0:S, :],
                                                in1=adps[:, 0:S, :],
                                                op=mybir.AluOpType.add)
                        lv = msb.tile([128, SMAX, 4], dt.float32, tag="lv")
                        nc.vector.tensor_scalar_mul(lv[:, 0:S, :],
                                                    ev[:, 0:S, :], NEG_SLOPE)
                        nc.vector.tensor_tensor(out=lv[:, 0:S, :],
                                                in0=ev[:, 0:S, :],
                                                in1=lv[:, 0:S, :],
                                                op=mybir.AluOpType.max)
                        msg = msb.tile([128, SMAX, 260], dt.bfloat16,
                                       tag="msg")
                        nc.scalar.activation(msg[:, 0:S, 256:260],
                                             lv[:, 0:S, :],
                                             mybir.ActivationFunctionType.Exp)
                        nc.vector.tensor_tensor(
                            out=msg[:, 0:S, 0:256].rearrange(
                                "p t (c h) -> p t c h", h=H),
                            in0=xg4[:],
                            in1=msg[:, 0:S, 256:260][:, :, None, :]
                                .to_broadcast([128, S, C, H]),
                            op=mybir.AluOpType.mult)
                        for t in range(S):
                            nc.tensor.matmul(
                                out=agg[:], lhsT=oh[:, t, :],
                                rhs=msg[:, t, :],
                                start=(cls == 0 and t == 0),
                                stop=(cls == 1 and t == S - 1))
                    # ---- finalize ----
                    den = fsb.tile([128, 4], dt.float32)
                    nc.vector.tensor_scalar_add(den[:], agg[:, 256:260], 1e-30)
                    rec = fsb.tile([128, 4], dt.float32)
                    nc.vector.reciprocal(rec[:], den[:])
                    gat_u = fsb.tile([128, 256], dt.float32)
                    nc.vector.tensor_tensor(out=gat_u[:], in0=agg[:, 0:256],
                                            in1=att_inv_sb[:],
                                            op=mybir.AluOpType.mult)
                    gat = fsb.tile([128, 256], dt.float32)
                    nc.vector.tensor_tensor(
                        out=gat[:].rearrange("p (c h) -> p c h", h=H),
                        in0=gat_u[:].rearrange("p (c h) -> p c h", h=H),
                        in1=rec[:, None, :].to_broadcast([128, C, H]),
                        op=mybir.AluOpType.mult)
                    fps = finps.tile([128, 64], dt.float32, space="PSUM")
                    gt = fsb.tile([128, 2, 128], dt.bfloat16)
                    for k in range(2):
                        pst = tps.tile([128, 128], dt.float32, space="PSUM",
                                       tag="pst")
                        nc.tensor.transpose(out=pst[:],
                                            in_=gat[:, k * 128:(k + 1) * 128],
                                            identity=identf_sb[:])
                        nc.scalar.activation(gt[:, k, :], pst[:],
                                             mybir.ActivationFunctionType.Relu,
                                             bias=bnsh_sb[:, k:k + 1],
                                             scale=bnsc_sb[:, k:k + 1])
                        nc.tensor.matmul(out=fps[:], lhsT=gt[:, k, :],
                                         rhs=linw_sb[:, k, :],
                                         start=(k == 0), stop=(k == 1))
                    ob = fsb.tile([128, 64], dt.float32)
                    nc.vector.tensor_tensor(out=ob[:], in0=fps[:],
                                            in1=linb_sb[:],
                                            op=mybir.AluOpType.add)
                    nc.sync.dma_start(
                        out=out_dram[b * 128:(b + 1) * 128, :], in_=ob[:])
    nc.compile()
    return nc


# block-row unpermute: psum row r -> node offset (2r if r<64 else 2(r-64)+1)
_ROW_OF_SLOT = np.empty(128, dtype=np.int64)
_ROW_OF_SLOT[:64] = np.arange(64) * 2
_ROW_OF_SLOT[64:] = np.arange(64) * 2 + 1
_UNPERM = np.argsort(_ROW_OF_SLOT)  # node offset q -> slot index


def _install_ntff_shim():
    """Install the axon NTFF profiling hook (missing antenv.axon_hooks shim)."""
    import sys, types
    if "antenv.axon_hooks" in sys.modules:
        return
    m = types.ModuleType("antenv.axon_hooks")
    _h = [None]
    m.set_axon_ntff_profile_hook = lambda h: _h.__setitem__(0, h)
    m.get_axon_ntff_profile_hook = lambda: _h[0]
    sys.modules["antenv.axon_hooks"] = m
    import antenv
    antenv.axon_hooks = m
    from trn_agent_boot.trn_boot import _ntff_profile_via_ctypes
    hook = _ntff_profile_via_ctypes("/opt/axon/libaxon_pjrt.so")
    if hook is not None:
        m.set_axon_ntff_profile_hook(hook)


def kernel(**inputs):
    global LAST_EXEC_NS, LAST_RESULTS
    import os
    from concourse import bass_utils

    trace = os.environ.get("KERNEL_TRACE") == "1"
    if trace:
        try:
            _install_ntff_shim()
            bass_utils.upload_artifacts = lambda tmpdir: "(upload skipped)"
        except Exception as e:
            print("ntff shim failed:", e)
            trace = False

    idx_all, dst_all, dT_all, own_all, subt_cfg = _prep_edges(
        np.asarray(inputs["edge_index"]))
    params = _prep_params(
        inputs["x"], inputs["W"], inputs["att_src"], inputs["att_dst"],
        inputs["gat_bias"], inputs["bn_gamma"], inputs["bn_beta"],
        inputs["bn_mean"], inputs["bn_var"], inputs["lin_W"], inputs["lin_b"])

    nc = _build(subt_cfg)

    shared = dict(
        xT_t=params["xT_t"], wt_ext=params["wt_ext"], att_inv=params["att_inv"],
        ratio=params["ratio"], bnsc=params["bnsc"], bnsh=params["bnsh"],
        linw=params["linw"], linb=params["linb"], iota=params["iota"],
        iotap=params["iotap"], ident_f32=params["ident_f32"])
    in_maps = []
    for p in range(NCORES):
        m = dict(shared)
        m["idx"] = np.ascontiguousarray(idx_all[p])
        m["dstm"] = np.ascontiguousarray(dst_all[p])
        m["dstmT"] = np.ascontiguousarray(dT_all[p])
        m["ownidx"] = np.ascontiguousarray(own_all[p])
        in_maps.append(m)

    run_kwargs = {}
    if trace:
        run_kwargs = dict(trace=True, tmpdir=os.environ.get(
            "KERNEL_TRACE_DIR", "/tmp/gat_prof"))
        os.makedirs(run_kwargs["tmpdir"], exist_ok=True)
    res = bass_utils.run_bass_kernel_spmd(
        nc, in_maps, core_ids=list(range(NCORES)), **run_kwargs)
    LAST_EXEC_NS = res.exec_time_ns
    LAST_RESULTS = res

    full = np.empty((NPAD, 64), dtype=np.float32)
    for p in range(NCORES):
        blocks = res.results[p]["out"].reshape(NB, 128, 64)
        full[p * OWN:(p + 1) * OWN] = blocks[:, _UNPERM, :].reshape(OWN, 64)
    return full[:N]



# revision 2
# speedup vs baseline: 1.6309x; 1.6309x over previous
"""GAT layer (gnn_message_passing) on 8 Trainium2 NeuronCores — V2.

Strategy (dst-partitioned, replicated projection, rotated local tables):
  * Core p owns dst nodes [p*6272, (p+1)*6272) = 49 blocks of 128.
  * Every core computes the full projected table xp = x @ W.T (bf16,
    feature-permuted j = c*4+h, pre-scaled by att_src) PLUS an 8-col
    attention sidecar (a_src, a_dst per node — att vectors folded into the
    projection weights, so they come straight out of the matmul).
    The table is stored ROTATED per core: local row r holds global node
    (p*6272 + r) mod 50176, so each core's own dst rows are local rows
    0..6271 with core-independent addressing.
  * Edges (no self loops) are bucketed per (dst-block, src-parity) and
    gathered per cell via gpsimd dma_gather (512B rows, superrow int16
    indices). Gathers are ROTATED across the 4 SWDGE queues so descriptor
    generation runs on all four Q7 cpu pairs concurrently (~4x).
  * Per cell: one-hot(dst-slot) matmuls aggregate sum_e w*xp[src] and the
    softmax denominator into a per-block PSUM accumulator, with
    w = exp(leaky(a_src + a_dst)); a_src from head-wise row sums of the
    pre-scaled gathered row, a_dst via one-hot-transpose matmul against the
    block's sidecar column.
  * Self loops never enter the edge stream: the block's own rows (contiguous
    local rows b*128..b*128+127) and sidecar give msg_self, added during
    finalize before normalization.
  * Finalize: normalize, transpose, fused BN+bias (att_src unscale folded
    into the BN scale) + ReLU, final linear -> [6272, 64] per core.
"""

import numpy as np
import ml_dtypes

BF16 = ml_dtypes.bfloat16

N, E, F, H, C = 50000, 800000, 256, 4, 64
NEG_SLOPE = 0.2
BN_EPS = 1e-5
NCORES = 8
BLK = 128
NB = 49
OWN = NB * BLK           # 6272
NPAD = NCORES * OWN      # 50176
NT = NPAD // 128         # 392
NTC = NT // 4            # 98 chunks of 4 tiles
NSUP = NPAD // 2         # 25088 superrows

# feature permutation: new index j = c*4 + h  <->  old index h*64 + c
_OLD_OF_NEW = (np.arange(F) % H) * C + (np.arange(F) // H)

LAST_EXEC_NS = None
LAST_RESULTS = None


def _prep_edges(edge_index):
    src = np.asarray(edge_index[0], dtype=np.int64)
    dst = np.asarray(edge_index[1], dtype=np.int64)

    core = dst // OWN
    dst_local = dst - core * OWN
    block = dst_local // BLK
    dst_slot = (dst_local % BLK).astype(np.float32)    # identity slot map
    # rotated local row of src on the owning core; parity preserved
    rot = (src[None, :] - (np.arange(NCORES) * OWN)[:, None]) % NPAD  # [8, E]
    cls = (src % 2).astype(np.int64)
    gidx_rot = rot // 2                                # [8, E] superrow per core

    ncell = NB * 2
    cell = core * ncell + block * 2 + cls
    counts = np.bincount(cell, minlength=NCORES * ncell).reshape(NCORES, ncell)
    nie_list = [int(np.ceil(counts[:, ci].max() / 16)) * 16
                for ci in range(ncell)]
    subt_list = [(n + 127) // 128 for n in nie_list]
    ni_list = [s * 128 for s in subt_list]
    offs = np.zeros(ncell + 1, dtype=np.int64)
    np.cumsum(ni_list, out=offs[1:])
    TOT = int(offs[-1])

    order = np.argsort(cell, kind="stable")
    sorted_cell = cell[order]
    cell_starts = np.zeros(NCORES * ncell + 1, dtype=np.int64)
    np.cumsum(counts.reshape(-1), out=cell_starts[1:])
    rank = np.arange(len(order)) - cell_starts[sorted_cell]
    ci_of = sorted_cell % ncell
    core_of = sorted_cell // ncell
    flat_pos = offs[ci_of] + rank                      # position within core

    gidx_pad = np.zeros((NCORES, TOT), dtype=np.int64)
    gidx_pad[core_of, flat_pos] = gidx_rot[core_of, order]
    dstm_pad = np.full((NCORES, TOT), 200.0, dtype=np.float32)
    dstm_pad[core_of, flat_pos] = dst_slot[order]

    # wrapped gather indices [16, TOT//16] -> replicated x8 across partitions
    g = gidx_pad.astype(np.int16).reshape(NCORES, TOT // 16, 16)
    g = np.ascontiguousarray(g.transpose(0, 2, 1))
    idx_all = np.tile(g, (1, 8, 1))                    # [8, 128, TOT//16]

    # dst-slot stream in gather layout (edge i at [i%128, i//128])
    dst_w = np.empty((NCORES, 128, TOT // 128), dtype=BF16)
    for ci in range(ncell):
        seg = dstm_pad[:, offs[ci]:offs[ci + 1]].reshape(
            NCORES, subt_list[ci], 128)
        dst_w[:, :, offs[ci] // 128:offs[ci + 1] // 128] = \
            seg.transpose(0, 2, 1).astype(BF16)
    dT_all = dstm_pad.astype(BF16)                     # [8, TOT] free-stream

    return idx_all, dst_w, dT_all, (subt_list, nie_list)


def _prep_params(x, W, att_src, att_dst, gat_bias, bn_gamma, bn_beta,
                 bn_mean, bn_var, lin_W, lin_b):
    f32 = np.float32
    W = np.asarray(W, f32)
    att_src_f = np.asarray(att_src, f32).reshape(H * C)      # index h*64+c
    att_src_hc = np.asarray(att_src, f32)                    # [H, C]
    att_dst_hc = np.asarray(att_dst, f32)

    wt = W.T                                                 # [in, out_old]
    wt_perm = wt[:, _OLD_OF_NEW] * att_src_f[_OLD_OF_NEW][None, :]
    # attention sidecar columns: a_src/a_dst = x @ (W_h.T @ att_h)
    aw_src = np.zeros((F, H), dtype=f32)
    aw_dst = np.zeros((F, H), dtype=f32)
    for h in range(H):
        aw_src[:, h] = W[h * C:(h + 1) * C, :].T @ att_src_hc[h]
        aw_dst[:, h] = W[h * C:(h + 1) * C, :].T @ att_dst_hc[h]
    wt_full = np.concatenate([wt_perm, aw_src, aw_dst], axis=1)  # [256, 264]
    wt_ext = np.ascontiguousarray(wt_full.reshape(2, 128, 264)).astype(BF16)

    # x transposed, padded, 4-tile-chunked partition-major
    xT = np.zeros((F, NPAD), dtype=f32)
    xT[:, :N] = np.asarray(x, f32).T
    xT_t = np.ascontiguousarray(
        xT.reshape(2, 128, NT, 128).transpose(2, 1, 0, 3)).astype(BF16)
    # [NT, 128, 2, 128] -> rotated per core later; chunking done per core

    bnscale = np.asarray(bn_gamma, f32) / np.sqrt(np.asarray(bn_var, f32) + BN_EPS)
    bnshift = ((np.asarray(gat_bias, f32) - np.asarray(bn_mean, f32)) * bnscale
               + np.asarray(bn_beta, f32))
    bnsc_f = bnscale[_OLD_OF_NEW] / att_src_f[_OLD_OF_NEW]   # fold unscale
    bnsc = np.ascontiguousarray(bnsc_f.reshape(2, 128).T)
    bnsh = np.ascontiguousarray(bnshift[_OLD_OF_NEW].reshape(2, 128).T)

    linw = np.asarray(lin_W, f32).T[_OLD_OF_NEW, :]
    linw_t = np.ascontiguousarray(linw.reshape(2, 128, 64)).astype(BF16)
    linb_rep = np.tile(np.asarray(lin_b, f32)[None, :], (128, 1))

    iota_row = np.tile(np.arange(128, dtype=np.float32)[None, :],
                       (128, 1)).astype(BF16)
    iota_p = np.arange(128, dtype=np.float32).reshape(128, 1).astype(BF16)
    ident_f32 = np.eye(128, dtype=np.float32)

    return dict(xT_t=xT_t, wt_ext=wt_ext, bnsc=bnsc.astype(f32),
                bnsh=bnsh.astype(f32), linw=linw_t, linb=linb_rep.astype(f32),
                iota=iota_row, iotap=iota_p, ident_f32=ident_f32)


def _build(subt_cfg):
    import concourse.bacc as bacc
    import concourse.mybir as mybir
    import concourse.tile as tile

    dt = mybir.dt
    subt_list, nie_list = subt_cfg
    NCH = NB * 2
    ni_list = [s * 128 for s in subt_list]
    offs = [0]
    for n in ni_list:
        offs.append(offs[-1] + n)
    TOT = offs[-1]
    SMAX = max(subt_list)

    nc = bacc.Bacc("TRN2", target_bir_lowering=False, debug=False,
                   enable_asserts=False, num_devices=NCORES,
                   num_swdge_queues=4)

    xT4_in = nc.dram_tensor("xT4", [NTC, 128, 4, 2, 128], dt.bfloat16,
                            kind="ExternalInput")
    wt_in = nc.dram_tensor("wt_ext", [2, 128, 264], dt.bfloat16,
                           kind="ExternalInput")
    bnsc_in = nc.dram_tensor("bnsc", [128, 2], dt.float32, kind="ExternalInput")
    bnsh_in = nc.dram_tensor("bnsh", [128, 2], dt.float32, kind="ExternalInput")
    linw_in = nc.dram_tensor("linw", [2, 128, 64], dt.bfloat16, kind="ExternalInput")
    linb_in = nc.dram_tensor("linb", [128, 64], dt.float32, kind="ExternalInput")
    iota_in = nc.dram_tensor("iota", [128, 128], dt.bfloat16, kind="ExternalInput")
    iotap_in = nc.dram_tensor("iotap", [128, 1], dt.bfloat16, kind="ExternalInput")
    identf_in = nc.dram_tensor("ident_f32", [128, 128], dt.float32, kind="ExternalInput")
    idx_in = nc.dram_tensor("idx", [128, TOT // 16], dt.int16, kind="ExternalInput")
    dstm_in = nc.dram_tensor("dstm", [128, TOT // 128], dt.bfloat16, kind="ExternalInput")
    dstmT_in = nc.dram_tensor("dstmT", [TOT], dt.bfloat16, kind="ExternalInput")
    out_dram = nc.dram_tensor("out", [OWN, 64], dt.float32, kind="ExternalOutput")

    with tile.TileContext(nc) as tc:
        with (
            tc.tile_pool(name="dram", bufs=1, space="DRAM") as dramp,
            tc.tile_pool(name="const", bufs=1) as constp,
        ):
            xp_tab = dramp.tile([NPAD, 256], dt.bfloat16)
            att_tab = dramp.tile([NPAD, 8], dt.bfloat16)
            sup = xp_tab[:].rearrange("(s two) f -> s (two f)", two=2)
            tabw = xp_tab[:].rearrange("(c j p) f -> c p j f", j=4, p=128)
            attw = att_tab[:].rearrange("(c j p) f -> c p j f", j=4, p=128)

            # ---- consts (issue early; overlap with phase A) ----
            wt_sb = constp.tile([128, 2, 264], dt.bfloat16)
            for k in range(2):
                nc.sync.dma_start(out=wt_sb[:, k, :], in_=wt_in[k])
            idx_sb = constp.tile([128, TOT // 16], dt.int16)
            nc.sync.dma_start(out=idx_sb[:], in_=idx_in[:])
            dstm_sb = constp.tile([128, TOT // 128], dt.bfloat16)
            nc.sync.dma_start(out=dstm_sb[:], in_=dstm_in[:])
            bnsc_sb = constp.tile([128, 2], dt.float32)
            nc.sync.dma_start(out=bnsc_sb[:], in_=bnsc_in[:])
            bnsh_sb = constp.tile([128, 2], dt.float32)
            nc.sync.dma_start(out=bnsh_sb[:], in_=bnsh_in[:])
            linw_sb = constp.tile([128, 2, 64], dt.bfloat16)
            for k in range(2):
                nc.sync.dma_start(out=linw_sb[:, k, :], in_=linw_in[k])
            linb_sb = constp.tile([128, 64], dt.float32)
            nc.sync.dma_start(out=linb_sb[:], in_=linb_in[:])
            iota_sb = constp.tile([128, 128], dt.bfloat16)
            nc.sync.dma_start(out=iota_sb[:], in_=iota_in[:])
            iotap_sb = constp.tile([128, 1], dt.bfloat16)
            nc.sync.dma_start(out=iotap_sb[:], in_=iotap_in[:])
            identf_sb = constp.tile([128, 128], dt.float32)
            nc.sync.dma_start(out=identf_sb[:], in_=identf_in[:])

            # ---- phase A: projection (4 tiles per chunk) ----
            with (
                tc.tile_pool(name="proj_sb", bufs=3) as psb,
                tc.tile_pool(name="proj_out", bufs=3) as pxp,
                tc.tile_pool(name="proj_ps", bufs=4, space="PSUM") as pps,
            ):
                for c in range(NTC):
                    xt = psb.tile([128, 4, 2, 128], dt.bfloat16)
                    nc.sync.dma_start(out=xt[:], in_=xT4_in[c])
                    xp4 = pxp.tile([128, 4, 264], dt.bfloat16)
                    for j in range(4):
                        ps = pps.tile([128, 264], dt.float32, space="PSUM")
                        nc.tensor.matmul(out=ps[:], lhsT=xt[:, j, 0, :],
                                         rhs=wt_sb[:, 0, :],
                                         start=True, stop=False)
                        nc.tensor.matmul(out=ps[:], lhsT=xt[:, j, 1, :],
                                         rhs=wt_sb[:, 1, :],
                                         start=False, stop=True)
                        nc.scalar.activation(
                            xp4[:, j, :], ps[:],
                            mybir.ActivationFunctionType.Copy)
                    nc.sync.dma_start(out=tabw[c], in_=xp4[:, :, 0:256])
                    nc.sync.dma_start(out=attw[c], in_=xp4[:, :, 256:264])

            # ---- phase B: per-block pipeline ----
            with (
                tc.tile_pool(name="gsb", bufs=4) as gsb,
                tc.tile_pool(name="ohsb", bufs=4) as ohsb,
                tc.tile_pool(name="msb", bufs=4) as msb,
                tc.tile_pool(name="osb", bufs=3) as osb,
                tc.tile_pool(name="fsb", bufs=2) as fsb,
                tc.tile_pool(name="aggps", bufs=3, space="PSUM") as aggps,
                tc.tile_pool(name="adstps", bufs=2, space="PSUM") as adstps,
                tc.tile_pool(name="tps", bufs=2, space="PSUM") as tps,
                tc.tile_pool(name="finps", bufs=1, space="PSUM") as finps,
            ):
                qctr = [0]

                def gq():
                    q = qctr[0] % 4
                    qctr[0] += 1
                    return q

                for b in range(NB):
                    own_x = osb.tile([128, 256], dt.bfloat16, tag="ox")
                    nc.sync.dma_start(
                        out=own_x[:], in_=xp_tab[b * 128:(b + 1) * 128, :])
                    own_a = osb.tile([128, 8], dt.bfloat16, tag="oa")
                    nc.sync.dma_start(
                        out=own_a[:], in_=att_tab[b * 128:(b + 1) * 128, :])

                    agg = aggps.tile([128, 260], dt.float32, space="PSUM")
                    for cls in range(2):
                        ci = b * 2 + cls
                        S = subt_list[ci]
                        NI = ni_list[ci]
                        oE = offs[ci]
                        oW = oE // 16
                        oS = oE // 128
                        nie = nie_list[ci]
                        xg = gsb.tile([128, SMAX, 256], dt.bfloat16,
                                      tag=f"xg{cls}")
                        src_ap = sup[:, 0:256] if cls == 0 else sup[:, 256:512]
                        if nie < S * 128:
                            nc.gpsimd.memset(xg[:, S - 1, :], 0.0)
                        for g0 in range(0, S, 8):
                            nrem = min(nie - g0 * 128, 1024)
                            if nrem <= 0:
                                break
                            gs = (nrem + 127) // 128
                            nc.gpsimd.dma_gather(
                                out_ap=xg[:, g0:g0 + gs, :], in_ap=src_ap,
                                idxs_ap=idx_sb[:, oW + g0 * 8:
                                               oW + g0 * 8 + (nrem + 15) // 16],
                                num_idxs=nrem, num_idxs_reg=nrem,
                                elem_size=256, elem_step=512, queue_num=gq())
                        # one-hot [edge_p, dst] and transposed [dst_p, edge]
                        oh = ohsb.tile([128, SMAX, 128], dt.bfloat16, tag="oh")
                        nc.vector.tensor_tensor(
                            out=oh[:, 0:S, :],
                            in0=dstm_sb[:, oS:oS + S, None].to_broadcast(
                                [128, S, 128]),
                            in1=iota_sb[:, None, :].to_broadcast(
                                [128, S, 128]),
                            op=mybir.AluOpType.is_equal)
                        dT = ohsb.tile([128, SMAX, 128], dt.bfloat16, tag="dT")
                        nc.scalar.dma_start(
                            out=dT[:, 0:S, :],
                            in_=dstmT_in[None, oE:oE + NI].to_broadcast(
                                [128, NI]))
                        ohT = ohsb.tile([128, SMAX, 128], dt.bfloat16, tag="ohT")
                        nc.vector.tensor_tensor(
                            out=ohT[:, 0:S, :], in0=dT[:, 0:S, :],
                            in1=iotap_sb[:, :, None].to_broadcast(
                                [128, S, 128]),
                            op=mybir.AluOpType.is_equal)
                        # a_src: head-wise row sums via 2x-mode tree adds
                        xg4 = xg[:, 0:S, :].rearrange(
                            "p t (c h) -> p t c h", h=H)
                        tr1 = msb.tile([128, SMAX, 32, 4], dt.bfloat16,
                                       tag="tr1")
                        nc.vector.tensor_tensor(
                            out=tr1[:, 0:S, :, :], in0=xg4[:, :, 0:32, :],
                            in1=xg4[:, :, 32:64, :], op=mybir.AluOpType.add)
                        tr2 = msb.tile([128, SMAX, 16, 4], dt.bfloat16,
                                       tag="tr2")
                        nc.vector.tensor_tensor(
                            out=tr2[:, 0:S, :, :], in0=tr1[:, 0:S, 0:16, :],
                            in1=tr1[:, 0:S, 16:32, :], op=mybir.AluOpType.add)
                        tr3 = msb.tile([128, SMAX, 8, 4], dt.bfloat16,
                                       tag="tr3")
                        nc.vector.tensor_tensor(
                            out=tr3[:, 0:S, :, :], in0=tr2[:, 0:S, 0:8, :],
                            in1=tr2[:, 0:S, 8:16, :], op=mybir.AluOpType.add)
                        asrc = msb.tile([128, SMAX, 4], dt.float32, tag="asrc")
                        nc.vector.reduce_sum(
                            out=asrc[:, 0:S, :],
                            in_=tr3[:, 0:S, :, :].rearrange(
                                "p t c h -> p t h c"),
                            axis=mybir.AxisListType.X)
                        # a_dst per edge via one-hot-transpose matmuls
                        adps = adstps.tile([128, SMAX, 4], dt.float32,
                                           space="PSUM")
                        for t in range(S):
                            nc.tensor.matmul(out=adps[:, t, :],
                                             lhsT=ohT[:, t, :],
                                             rhs=own_a[:, 4:8],
                                             start=True, stop=True)
                        # w = exp(leaky(a_src + a_dst)) -> msg cols 256:260
                        ev = msb.tile([128, SMAX, 4], dt.float32, tag="ev")
                        nc.vector.tensor_tensor(out=ev[:, 0:S, :],
                                                in0=asrc[:, 0:S, :],
                                                in1=adps[:, 0:S, :],
                                                op=mybir.AluOpType.add)
                        lv = msb.tile([128, SMAX, 4], dt.float32, tag="lv")
                        nc.vector.scalar_tensor_tensor(
                            out=lv[:, 0:S, :], in0=ev[:, 0:S, :],
                            scalar=NEG_SLOPE, in1=ev[:, 0:S, :],
                            op0=mybir.AluOpType.mult,
                            op1=mybir.AluOpType.max)
                        msg = msb.tile([128, SMAX, 260], dt.bfloat16,
                                       tag="msg")
                        nc.scalar.activation(msg[:, 0:S, 256:260],
                                             lv[:, 0:S, :],
                                             mybir.ActivationFunctionType.Exp)
                        nc.vector.tensor_tensor(
                            out=msg[:, 0:S, 0:256].rearrange(
                                "p t (c h) -> p t c h", h=H),
                            in0=xg4[:],
                            in1=msg[:, 0:S, 256:260][:, :, None, :]
                                .to_broadcast([128, S, C, H]),
                            op=mybir.AluOpType.mult)
                        for t in range(S):
                            nc.tensor.matmul(
                                out=agg[:], lhsT=oh[:, t, :],
                                rhs=msg[:, t, :],
                                start=(cls == 0 and t == 0),
                                stop=(cls == 1 and t == S - 1))
                    # ---- finalize (self loop + normalize + BN + linear) ----
                    evs = fsb.tile([128, 4], dt.float32, tag="evs")
                    nc.vector.tensor_tensor(out=evs[:], in0=own_a[:, 0:4],
                                            in1=own_a[:, 4:8],
                                            op=mybir.AluOpType.add)
                    lvs = fsb.tile([128, 4], dt.float32, tag="lvs")
                    nc.vector.scalar_tensor_tensor(
                        out=lvs[:], in0=evs[:], scalar=NEG_SLOPE, in1=evs[:],
                        op0=mybir.AluOpType.mult, op1=mybir.AluOpType.max)
                    selfmsg = fsb.tile([128, 260], dt.float32, tag="sm")
                    nc.scalar.activation(selfmsg[:, 256:260], lvs[:],
                                         mybir.ActivationFunctionType.Exp)
                    nc.vector.tensor_tensor(
                        out=selfmsg[:, 0:256].rearrange(
                            "p (c h) -> p c h", h=H),
                        in0=own_x[:].rearrange("p (c h) -> p c h", h=H),
                        in1=selfmsg[:, 256:260][:, None, :].to_broadcast(
                            [128, C, H]),
                        op=mybir.AluOpType.mult)
                    tot = fsb.tile([128, 260], dt.float32, tag="tot")
                    nc.vector.tensor_tensor(out=tot[:], in0=agg[:],
                                            in1=selfmsg[:],
                                            op=mybir.AluOpType.add)
                    rec = fsb.tile([128, 4], dt.float32, tag="rec")
                    nc.vector.reciprocal(rec[:], tot[:, 256:260])
                    gat = fsb.tile([128, 256], dt.float32, tag="gat")
                    nc.vector.tensor_tensor(
                        out=gat[:].rearrange("p (c h) -> p c h", h=H),
                        in0=tot[:, 0:256].rearrange("p (c h) -> p c h", h=H),
                        in1=rec[:, None, :].to_broadcast([128, C, H]),
                        op=mybir.AluOpType.mult)
                    fps = finps.tile([128, 64], dt.float32, space="PSUM")
                    gt = fsb.tile([128, 2, 128], dt.bfloat16, tag="gt")
                    for k in range(2):
                        pst = tps.tile([128, 128], dt.float32, space="PSUM",
                                       tag="pst")
                        nc.tensor.transpose(out=pst[:],
                                            in_=gat[:, k * 128:(k + 1) * 128],
                                            identity=identf_sb[:])
                        nc.scalar.activation(gt[:, k, :], pst[:],
                                             mybir.ActivationFunctionType.Relu,
                                             bias=bnsh_sb[:, k:k + 1],
                                             scale=bnsc_sb[:, k:k + 1])
                        nc.tensor.matmul(out=fps[:], lhsT=gt[:, k, :],
                                         rhs=linw_sb[:, k, :],
                                         start=(k == 0), stop=(k == 1))
                    ob = fsb.tile([128, 64], dt.float32, tag="ob")
                    nc.vector.tensor_tensor(out=ob[:], in0=fps[:],
                                            in1=linb_sb[:],
                                            op=mybir.AluOpType.add)
                    nc.sync.dma_start(
                        out=out_dram[b * 128:(b + 1) * 128, :], in_=ob[:])
    nc.compile()
    return nc


def _install_ntff_shim():
    """Install the axon NTFF profiling hook (missing antenv.axon_hooks shim)."""
    import sys, types
    if "antenv.axon_hooks" in sys.modules:
        return
    m = types.ModuleType("antenv.axon_hooks")
    _h = [None]
    m.set_axon_ntff_profile_hook = lambda h: _h.__setitem__(0, h)
    m.get_axon_ntff_profile_hook = lambda: _h[0]
    sys.modules["antenv.axon_hooks"] = m
    import antenv
    antenv.axon_hooks = m
    from trn_agent_boot.trn_boot import _ntff_profile_via_ctypes
    hook = _ntff_profile_via_ctypes("/opt/axon/libaxon_pjrt.so")
    if hook is not None:
        m.set_axon_ntff_profile_hook(hook)


def kernel(**inputs):
    global LAST_EXEC_NS, LAST_RESULTS
    import os
    from concourse import bass_utils

    trace = os.environ.get("KERNEL_TRACE") == "1"
    if trace:
        try:
            _install_ntff_shim()
            bass_utils.upload_artifacts = lambda tmpdir: "(upload skipped)"
        except Exception as e:
            print("ntff shim failed:", e)
            trace = False

    idx_all, dst_all, dT_all, subt_cfg = _prep_edges(
        np.asarray(inputs["edge_index"]))
    params = _prep_params(
        inputs["x"], inputs["W"], inputs["att_src"], inputs["att_dst"],
        inputs["gat_bias"], inputs["bn_gamma"], inputs["bn_beta"],
        inputs["bn_mean"], inputs["bn_var"], inputs["lin_W"], inputs["lin_b"])

    nc = _build(subt_cfg)

    xT_t = params["xT_t"]                    # [NT, 128, 2, 128]
    shared = dict(
        wt_ext=params["wt_ext"], bnsc=params["bnsc"], bnsh=params["bnsh"],
        linw=params["linw"], linb=params["linb"], iota=params["iota"],
        iotap=params["iotap"], ident_f32=params["ident_f32"])
    in_maps = []
    for p in range(NCORES):
        m = dict(shared)
        rot = np.roll(np.arange(NT), -p * NB)     # tile t holds local rows
        xr = xT_t[rot]                            # [NT, 128, 2, 128]
        m["xT4"] = np.ascontiguousarray(
            xr.reshape(NTC, 4, 128, 2, 128).transpose(0, 2, 1, 3, 4))
        m["idx"] = np.ascontiguousarray(idx_all[p])
        m["dstm"] = np.ascontiguousarray(dst_all[p])
        m["dstmT"] = np.ascontiguousarray(dT_all[p])
        in_maps.append(m)

    run_kwargs = {}
    if trace:
        run_kwargs = dict(trace=True, tmpdir=os.environ.get(
            "KERNEL_TRACE_DIR", "/tmp/gat_prof"))
        os.makedirs(run_kwargs["tmpdir"], exist_ok=True)
    res = bass_utils.run_bass_kernel_spmd(
        nc, in_maps, core_ids=list(range(NCORES)), **run_kwargs)
    LAST_EXEC_NS = res.exec_time_ns
    LAST_RESULTS = res

    full = np.empty((NPAD, 64), dtype=np.float32)
    for p in range(NCORES):
        full[p * OWN:(p + 1) * OWN] = res.results[p]["out"]
    return full[:N]
